# revision 24
# baseline (speedup 1.0000x reference)
"""Fused multi-head attention block (B=2, N=4096, C=768, H=12, D=64) for 8
Trainium2 NeuronCores — v3.

Sharding: core c -> (batch b = c // 4, head-group g = c % 4, heads
[3g, 3g+1, 3g+2]).  Megatron-style: qkv weights column-split per head
group, proj weights row-split; each core emits a partial [N, C] output
and the host sums the 4 partials per batch and adds proj_b.

Design:
  - Scores for the 3 heads land interleaved in one [128, 1536] PSUM tile
    (3 banks, double buffered = 6 banks), so one exp instruction covers
    all 3 heads of a key block (amortizes ACT per-instruction overhead).
  - exp split across engines: most key blocks on ACT (Exp, bf16 out), a
    tunable subset on DVE via a Schraudolph bit trick: i16 = int(s*128*
    log2e + (127*128 - c)), bitcast bf16 == 2^(s*log2e) with ~1.8%
    multiplicative noise that cancels through softmax normalization.
  - AV reoriented: oacc[128 q, 65] += pt_blk^T @ vaug_blk runs the PE
    contraction at full 128 with 128 output rows; col 64 of vaug is 1.0
    so col 64 of oacc accumulates the softmax denominator.  The 12
    (q-subtile, head) accumulators pack 6-per-PSUM-bank with one
    accumulation group per bank (start only on the first window write,
    stop on the last; first writes overwrite-from-pending-zero).
  - Normalization on ACT (activation Copy with per-partition scale),
    reciprocals via one strided DVE reciprocal per bank.  gpsimd cannot
    access PSUM on TRN2, so ACT/DVE carry all PSUM reads.
  - O transposed back to [d, q] with identity matmuls; heads a+b pack
    into one 128-partition tile so proj contracts two heads per matmul
    (Megatron row-split); head c runs at K=64.  NOTE: matmul operands at
    partition offset 64 hang the PE, so the a|b v-transpose uses the full
    128-partition vst01 against eye128 (out cols 0:64=a, 64:128=b).
  - Aggressive cross-chunk software pipelining: phase 1 (k/v proj +
    v-transpose) for key-chunks 1..7 is emitted inside query-chunk 0's
    kb loop (ACT is otherwise idle during phase 1), and each chunk's
    transpose/proj/y-DMA tail is emitted inside the NEXT chunk's kb loop
    so the PE never drains on the cross-engine normalize handoff.
  - K/Q stored bf16 (halves their SBUF, faster PE weight loads); V path
    bf16; qkv projection itself runs in f32r from f32 x and weights.
"""

import sys

sys.path.insert(0, "/opt/trn_rl_repo")

from contextlib import ExitStack

import numpy as np

import concourse.bacc as bacc
import concourse.bass as bass
import concourse.mybir as mybir
import concourse.tile as tile

B, N, C, H, D = 2, 4096, 768, 12, 64
SCALE = D ** -0.5
F32 = mybir.dt.float32
BF16 = mybir.dt.bfloat16
I16 = mybir.dt.int16
MM_DT = mybir.dt.float32r

# DVE Schraudolph exp2 trick constants (bf16 bit layout: exponent at bit 7)
LOG2E = 1.4426950408889634
DVE_A = 128.0 * LOG2E
DVE_B = 127.0 * 128.0 - 7.2 + 0.5  # -c to zero mean rel err, +0.5 for floor

# key blocks whose exp runs on DVE instead of ACT: placed where the PE
# stream has extra inserted work (tail at kb6-7, qproj at kb30-31) so the
# slower DVE exp hides behind it
def _dve_kbs(NB):
    return frozenset((6, 7, NB - 2, NB - 1))

# qkv weight column layout: m0 q01 | m1 k01 | m2 q2 | m3 k2 | m4 v01 | m5 v2
MOFF = [0, 128, 256, 320, 384, 512]
MW = [128, 128, 64, 64, 128, 64]
WCOLS = 576

Exp = mybir.ActivationFunctionType.Exp
Copy = mybir.ActivationFunctionType.Copy


def build_nc(seq=N):
    NS = seq // 512  # 512-wide query chunks
    NB = seq // 128  # 128-wide key blocks
    dve_kbs = _dve_kbs(NB)

    nc = bacc.Bacc("TRN2", target_bir_lowering=False, debug=False, num_devices=8)
    xt = nc.dram_tensor("xt", [768, seq], MM_DT, kind="ExternalInput").ap()
    wqkv = nc.dram_tensor("wqkv", [768, WCOLS], MM_DT, kind="ExternalInput").ap()
    wb = nc.dram_tensor("wb", [128, 6], F32, kind="ExternalInput").ap()
    pwt = nc.dram_tensor("pwt", [256, 768], F32, kind="ExternalInput").ap()
    ident = nc.dram_tensor("ident", [128, 192], F32, kind="ExternalInput").ap()
    y = nc.dram_tensor("y", [seq, 768], F32, kind="ExternalOutput").ap()

    with tile.TileContext(nc) as tc, ExitStack() as ctx:
        const = ctx.enter_context(tc.tile_pool(name="const", bufs=1))
        big = ctx.enter_context(tc.tile_pool(name="big", bufs=1))
        stg = ctx.enter_context(tc.tile_pool(name="stg", bufs=2))
        xs_pool = ctx.enter_context(tc.tile_pool(name="xs", bufs=18))
        pt_pool = ctx.enter_context(tc.tile_pool(name="pt", bufs=4))
        vst_pool = ctx.enter_context(tc.tile_pool(name="vst", bufs=4))
        onab_pool = ctx.enter_context(tc.tile_pool(name="onab", bufs=2))
        onc_pool = ctx.enter_context(tc.tile_pool(name="onc", bufs=2))
        otab_pool = ctx.enter_context(tc.tile_pool(name="otab", bufs=2))
        otc_pool = ctx.enter_context(tc.tile_pool(name="otc", bufs=2))
        dnm_pool = ctx.enter_context(tc.tile_pool(name="dnm", bufs=2))
        ysb_pool = ctx.enter_context(tc.tile_pool(name="ysb", bufs=4))
        sp = ctx.enter_context(tc.tile_pool(name="sp", bufs=2, space="PSUM"))
        oa = ctx.enter_context(tc.tile_pool(name="oa", bufs=2, space="PSUM"))

        # ---- constants ----
        w_sb = []
        for cch in range(6):
            t = const.tile([128, WCOLS], MM_DT, tag=f"w{cch}", name=f"w{cch}")
            nc.sync.dma_start(t[:], wqkv[cch * 128:(cch + 1) * 128, :])
            w_sb.append(t)
        wb_sb = const.tile([128, 6], F32, tag="wb")
        nc.sync.dma_start(wb_sb[:], wb[:])

        # proj weights + identity: DMA f32 staging, convert to bf16 once
        pws = stg.tile([128, 768], F32, tag="stg", name="pws_ab")
        nc.sync.dma_start(pws[:], pwt[0:128, :])
        pwab = const.tile([128, 768], BF16, tag="pwab")
        nc.vector.tensor_copy(pwab[:], pws[:])
        pwsc = stg.tile([128, 768], F32, tag="stg", name="pws_c")
        nc.sync.dma_start(pwsc[:], pwt[128:256, :])
        pwc = const.tile([64, 768], BF16, tag="pwc")
        nc.vector.tensor_copy(pwc[:], pwsc[0:64, :])
        ids = stg.tile([128, 192], F32, tag="ids", name="ids")
        nc.sync.dma_start(ids[:], ident[:])
        idb = const.tile([128, 192], BF16, tag="idb")
        nc.vector.tensor_copy(idb[:], ids[:])

        # ---- persistent qkv^T tensors (bf16) ----
        ka = big.tile([128, seq], BF16, tag="ka")
        kb_ = big.tile([128, seq], BF16, tag="kb")
        kc = big.tile([128, seq], BF16, tag="kc")
        q01 = big.tile([128, seq], BF16, tag="q01")
        q2 = big.tile([128, seq], BF16, tag="q2")
        nc.vector.memset(ka[64:128, :], 0.0)
        nc.vector.memset(kb_[0:64, :], 0.0)
        nc.vector.memset(kc[64:128, :], 0.0)
        nc.vector.memset(q2[64:128, :], 0.0)
        vaug = [
            big.tile([128, NB * 65], BF16, tag=f"va{h}", name=f"va{h}")
            for h in range(3)
        ]
        for h in range(3):
            nc.vector.memset(vaug[h][:], 1.0)

        def qproj(Q):
            """Project q01/q2 for query chunk Q (12 matmuls + 2 copybacks)."""
            qs = slice(Q * 512, (Q + 1) * 512)
            xq = []
            for cch in range(6):
                t = xs_pool.tile([128, 512], MM_DT, tag="xs", name="xq")
                nc.sync.dma_start(t[:], xt[cch * 128:(cch + 1) * 128, qs])
                xq.append(t)
            spq = sp.tile([128, 1536], F32, tag="sp", name="spq")
            for m, co in ((0, 0), (2, 512)):
                w = MW[m]
                for cch in range(6):
                    nc.tensor.matmul(
                        spq[0:w, co:co + 512],
                        lhsT=w_sb[cch][:, MOFF[m]:MOFF[m] + w],
                        rhs=xq[cch][:],
                        start=(cch == 0),
                        stop=(cch == 5),
                    )
            nc.vector.tensor_scalar_add(q01[:, qs], spq[:, 0:512], wb_sb[:, 0:1])
            nc.vector.tensor_scalar_add(
                q2[0:64, qs], spq[0:64, 512:1024], wb_sb[0:64, 2:3]
            )

        def phase1(s):
            """k/v projection + v transpose for key chunk s (all via sp pool)."""
            ss = slice(s * 512, (s + 1) * 512)
            xs = []
            for cch in range(6):
                t = xs_pool.tile([128, 512], MM_DT, tag="xs", name="xs")
                nc.sync.dma_start(t[:], xt[cch * 128:(cch + 1) * 128, ss])
                xs.append(t)
            kps = sp.tile([128, 1536], F32, tag="sp", name="kps")
            for m, co in ((1, 0), (3, 512)):
                w = MW[m]
                for cch in range(6):
                    nc.tensor.matmul(
                        kps[0:w, co:co + 512],
                        lhsT=w_sb[cch][:, MOFF[m]:MOFF[m] + w],
                        rhs=xs[cch][:],
                        start=(cch == 0),
                        stop=(cch == 5),
                    )
            nc.vector.tensor_scalar_add(ka[0:64, ss], kps[0:64, 0:512],
                                        wb_sb[0:64, 1:2])
            nc.vector.tensor_scalar_add(kb_[64:128, ss], kps[64:128, 0:512],
                                        wb_sb[64:128, 1:2])
            nc.vector.tensor_scalar_add(kc[0:64, ss], kps[0:64, 512:1024],
                                        wb_sb[0:64, 3:4])
            vps = sp.tile([128, 1536], F32, tag="sp", name="vps")
            for m, co in ((4, 0), (5, 512)):
                w = MW[m]
                for cch in range(6):
                    nc.tensor.matmul(
                        vps[0:w, co:co + 512],
                        lhsT=w_sb[cch][:, MOFF[m]:MOFF[m] + w],
                        rhs=xs[cch][:],
                        start=(cch == 0),
                        stop=(cch == 5),
                    )
            vst01 = vst_pool.tile([128, 512], BF16, tag="vst01", name="vst01")
            nc.scalar.activation(vst01[:], vps[:, 0:512], Copy)
            vst2 = vst_pool.tile([64, 512], BF16, tag="vst2", name="vst2")
            nc.scalar.activation(vst2[:], vps[0:64, 512:1024], Copy)
            # transpose v into vaug [keys, d]: a|b via eye128 (cols split),
            # c via eye64.  One sp tile: a|b in cols 0:512, c in 1024:1280.
            tp = sp.tile([128, 1536], F32, tag="sp", name="tpv")
            for j in range(4):
                nc.tensor.matmul(
                    tp[:, j * 128:(j + 1) * 128],
                    lhsT=vst01[:, j * 128:(j + 1) * 128],
                    rhs=idb[:, 64:192],
                    start=True,
                    stop=True,
                )
                nc.tensor.matmul(
                    tp[:, 1024 + j * 64:1024 + (j + 1) * 64],
                    lhsT=vst2[:, j * 128:(j + 1) * 128],
                    rhs=idb[0:64, 0:64],
                    start=True,
                    stop=True,
                )
            for j in range(4):
                blk = 4 * s + j
                nc.vector.tensor_copy(
                    vaug[0][:, blk * 65:blk * 65 + 64],
                    tp[:, j * 128:j * 128 + 64],
                )
                nc.vector.tensor_copy(
                    vaug[1][:, blk * 65:blk * 65 + 64],
                    tp[:, j * 128 + 64:j * 128 + 128],
                )
                nc.scalar.activation(
                    vaug[2][:, blk * 65:blk * 65 + 64],
                    tp[:, 1024 + j * 64:1024 + j * 64 + 64], Copy,
                )

        # tail state carried across chunks for software pipelining
        tail = {}

        def emit_tail_pe(Q):
            """Transposes + proj + y DMA for chunk Q (emitted inside Q+1)."""
            onab, onc = tail.pop(Q)
            tpab = sp.tile([128, 1536], F32, tag="sp", name="tpab")
            for j in range(4):
                nc.tensor.matmul(
                    tpab[:, j * 128:(j + 1) * 128],
                    lhsT=onab[:, j * 128:(j + 1) * 128],
                    rhs=idb[:, 64:192],
                    start=True,
                    stop=True,
                )
            otab = otab_pool.tile([128, 512], BF16, tag="otab")
            nc.vector.tensor_copy(otab[:], tpab[:, 0:512])
            tpc = sp.tile([128, 1536], F32, tag="sp", name="tpc")
            for j in range(4):
                nc.tensor.matmul(
                    tpc[0:64, j * 128:(j + 1) * 128],
                    lhsT=onc[:, j * 64:(j + 1) * 64],
                    rhs=idb[:, 64:192],
                    start=True,
                    stop=True,
                )
            otc = otc_pool.tile([64, 512], BF16, tag="otc")
            nc.vector.tensor_copy(otc[:], tpc[0:64, 0:512])
            for j in range(4):
                psy = sp.tile([128, 1536], F32, tag="sp", name="psy")
                for co, cw in ((0, 512), (512, 256)):
                    nc.tensor.matmul(
                        psy[:, co:co + cw],
                        lhsT=otab[:, j * 128:(j + 1) * 128],
                        rhs=pwab[:, co:co + cw],
                        start=True,
                        stop=False,
                    )
                    nc.tensor.matmul(
                        psy[:, co:co + cw],
                        lhsT=otc[:, j * 128:(j + 1) * 128],
                        rhs=pwc[:, co:co + cw],
                        start=False,
                        stop=True,
                    )
                ysb = ysb_pool.tile([128, 768], F32, tag="ysb", name="ysb")
                nc.vector.tensor_copy(ysb[:], psy[:, 0:768])
                r0 = Q * 512 + j * 128
                nc.sync.dma_start(y[r0:r0 + 128, :], ysb[:])

        # ---- prologue: first key chunk + first q chunk ----
        phase1(0)
        qproj(0)

        # ---- main loop over query chunks ----
        for Q in range(NS):
            qs = slice(Q * 512, (Q + 1) * 512)
            oaA = oa.tile([128, 512], F32, tag="oa", name="oaA")
            oaB = oa.tile([128, 512], F32, tag="oa", name="oaB")

            def emit_av(kb, ptt, oaA=oaA, oaB=oaB):
                # One PSUM accumulation group per bank: start marks the whole
                # 2KB zero region; later windows' first writes overwrite from
                # pending-zero, so only window 0 starts and window 5 stops.
                for h in range(3):
                    for j in range(4):
                        idx = h * 4 + j
                        bank, pos = (oaA, idx) if idx < 6 else (oaB, idx - 6)
                        nc.tensor.matmul(
                            bank[:, pos * 65:pos * 65 + 65],
                            lhsT=ptt[:, h * 512 + j * 128:h * 512 + (j + 1) * 128],
                            rhs=vaug[h][:, kb * 65:(kb + 1) * 65],
                            start=(kb == 0 and pos == 0),
                            stop=(kb == NB - 1 and pos == 5),
                        )

            pend = []
            for kb in range(NB):
                spt = sp.tile([128, 1536], F32, tag="sp", name="spt")
                for h, (kt, qt) in enumerate(((ka, q01), (kb_, q01), (kc, q2))):
                    nc.tensor.matmul(
                        spt[:, h * 512:(h + 1) * 512],
                        lhsT=kt[:, kb * 128:(kb + 1) * 128],
                        rhs=qt[:, qs],
                        start=True,
                        stop=True,
                    )
                ptt = pt_pool.tile([128, 1536], BF16, tag="pt", name="pt")
                if kb in dve_kbs:
                    nc.vector.tensor_scalar(
                        ptt[:].bitcast(I16), spt[:], DVE_A, DVE_B,
                        mybir.AluOpType.mult, mybir.AluOpType.add,
                    )
                else:
                    nc.scalar.activation(ptt[:], spt[:], Exp)
                pend.append((kb, ptt))
                if len(pend) >= 3:
                    emit_av(*pend.pop(0))
                if Q == 0 and kb % 4 == 3 and kb < 4 * (NS - 1):
                    phase1(kb // 4 + 1)  # fuse remaining k/v chunks into Q0
                if Q > 0 and kb == 6:
                    emit_tail_pe(Q - 1)  # previous chunk's transposes + proj
            if Q + 1 < NS:
                qproj(Q + 1)
            for e in pend:
                emit_av(*e)

            # normalize at the chunk boundary: the next chunk's AV reuses
            # these oa banks (WAR), so split the 12 multiplies across DVE
            # (bank A) and ACT (bank B) to clear them within ~2 kbs.
            dnm = dnm_pool.tile([128, 16], F32, tag="dnm")
            nc.vector.reciprocal(dnm[:, 6:12], oaB[:, 64:64 + 6 * 65:65])
            nc.vector.reciprocal(dnm[:, 0:6], oaA[:, 64:64 + 6 * 65:65])
            onab = onab_pool.tile([128, 512], BF16, tag="onab")
            onc = onc_pool.tile([128, 256], BF16, tag="onc")
            for h in range(3):
                for j in range(4):
                    idx = h * 4 + j
                    bank, pos = (oaA, idx) if idx < 6 else (oaB, idx - 6)
                    if h < 2:
                        dst = onab[:, j * 128 + h * 64:j * 128 + (h + 1) * 64]
                    else:
                        dst = onc[:, j * 64:(j + 1) * 64]
                    if idx < 6:
                        nc.vector.tensor_scalar_mul(
                            dst, bank[:, pos * 65:pos * 65 + 64],
                            dnm[:, idx:idx + 1]
                        )
                    else:
                        nc.scalar.activation(
                            dst, bank[:, pos * 65:pos * 65 + 64], Copy,
                            scale=dnm[:, idx:idx + 1],
                        )
            tail[Q] = (onab, onc)
        emit_tail_pe(NS - 1)

    nc.compile()
    return nc


def host_prep(x, qkv_w, qkv_b, proj_w, seq=N):
    """Build the 8 per-core input maps."""
    f = np.float32
    x = np.asarray(x, f)
    qkv_w = np.asarray(qkv_w, f)
    qkv_b = np.asarray(qkv_b, f)
    proj_w = np.asarray(proj_w, f)

    xts = [np.ascontiguousarray(x[b].T) for b in range(B)]
    ident = np.zeros((128, 192), f)
    ident[0:64, 0:64] = np.eye(64, dtype=f)
    ident[64:128, 0:64] = np.eye(64, dtype=f)
    ident[:, 64:192] = np.eye(128, dtype=f)

    in_maps = []
    for core in range(8):
        b, g = core // 4, core % 4
        ha, hb_, hc = 3 * g, 3 * g + 1, 3 * g + 2

        def Wrow(base, h):
            return qkv_w[base + h * 64:base + (h + 1) * 64, :]  # [64, 768]

        def brow(base, h):
            return qkv_b[base + h * 64:base + (h + 1) * 64]

        cols = np.concatenate(
            [
                Wrow(0, ha).T * SCALE, Wrow(0, hb_).T * SCALE,  # m0 q01
                Wrow(C, ha).T, Wrow(C, hb_).T,                  # m1 k01
                Wrow(0, hc).T * SCALE,                          # m2 q2
                Wrow(C, hc).T,                                  # m3 k2
                Wrow(2 * C, ha).T, Wrow(2 * C, hb_).T,          # m4 v01
                Wrow(2 * C, hc).T,                              # m5 v2
            ],
            axis=1,
        )  # [768, 576]
        bias = np.concatenate(
            [
                brow(0, ha) * SCALE, brow(0, hb_) * SCALE,
                brow(C, ha), brow(C, hb_),
                brow(0, hc) * SCALE,
                brow(C, hc),
                brow(2 * C, ha), brow(2 * C, hb_), brow(2 * C, hc),
            ]
        )  # [576]
        wbm = np.zeros((128, 6), f)
        for m in range(6):
            wbm[0:MW[m], m] = bias[MOFF[m]:MOFF[m] + MW[m]]
        pwtm = np.zeros((256, 768), f)
        pwtm[0:64, :] = proj_w.T[ha * 64:(ha + 1) * 64, :]
        pwtm[64:128, :] = proj_w.T[hb_ * 64:(hb_ + 1) * 64, :]
        pwtm[128:192, :] = proj_w.T[hc * 64:(hc + 1) * 64, :]

        in_maps.append(
            {
                "xt": xts[b][:, :seq],
                "wqkv": np.ascontiguousarray(cols),
                "wb": wbm,
                "pwt": pwtm,
                "ident": ident,
            }
        )
    return in_maps


_nc_cache = {}


def _get_nc(seq=N):
    if seq not in _nc_cache:
        _nc_cache[seq] = build_nc(seq)
    return _nc_cache[seq]


def kernel(x, qkv_w, qkv_b, proj_w, proj_b, _trace=False):
    from concourse.bass_utils import run_bass_kernel_spmd

    nc = _get_nc()
    in_maps = host_prep(x, qkv_w, qkv_b, proj_w)
    res = run_bass_kernel_spmd(nc, in_maps, list(range(8)), trace=_trace)
    proj_b = np.asarray(proj_b, np.float32)
    out = np.zeros((B, N, C), np.float32)
    for b in range(B):
        acc = np.zeros((N, C), np.float32)
        for g in range(4):
            acc += res.results[b * 4 + g]["y"]
        out[b] = acc + proj_b[None, :]
    if _trace:
        return out, res
    return out


# revision 25
# speedup vs baseline: 1.0292x; 1.0292x over previous
"""Fused multi-head attention block (B=2, N=4096, C=768, H=12, D=64) for 8
Trainium2 NeuronCores — v3.

Sharding: core c -> (batch b = c // 4, head-group g = c % 4, heads
[3g, 3g+1, 3g+2]).  Megatron-style: qkv weights column-split per head
group, proj weights row-split; each core emits a partial [N, C] output
and the host sums the 4 partials per batch and adds proj_b.

Design:
  - Scores for the 3 heads land interleaved in one [128, 1536] PSUM tile
    (3 banks, double buffered = 6 banks), so one exp instruction covers
    all 3 heads of a key block (amortizes ACT per-instruction overhead).
  - exp split across engines: most key blocks on ACT (Exp, bf16 out), a
    tunable subset on DVE via a Schraudolph bit trick: i16 = int(s*128*
    log2e + (127*128 - c)), bitcast bf16 == 2^(s*log2e) with ~1.8%
    multiplicative noise that cancels through softmax normalization.
  - AV reoriented: oacc[128 q, 65] += pt_blk^T @ vaug_blk runs the PE
    contraction at full 128 with 128 output rows; col 64 of vaug is 1.0
    so col 64 of oacc accumulates the softmax denominator.  The 12
    (q-subtile, head) accumulators pack 6-per-PSUM-bank with one
    accumulation group per bank (start only on the first window write,
    stop on the last; first writes overwrite-from-pending-zero).
  - Normalization on ACT (activation Copy with per-partition scale),
    reciprocals via one strided DVE reciprocal per bank.  gpsimd cannot
    access PSUM on TRN2, so ACT/DVE carry all PSUM reads.
  - O transposed back to [d, q] with identity matmuls; heads a+b pack
    into one 128-partition tile so proj contracts two heads per matmul
    (Megatron row-split); head c runs at K=64.  NOTE: matmul operands at
    partition offset 64 hang the PE, so the a|b v-transpose uses the full
    128-partition vst01 against eye128 (out cols 0:64=a, 64:128=b).
  - Aggressive cross-chunk software pipelining: phase 1 (k/v proj +
    v-transpose) for key-chunks 1..7 is emitted inside query-chunk 0's
    kb loop (ACT is otherwise idle during phase 1), and each chunk's
    transpose/proj/y-DMA tail is emitted inside the NEXT chunk's kb loop
    so the PE never drains on the cross-engine normalize handoff.
  - K/Q stored bf16 (halves their SBUF, faster PE weight loads); V path
    bf16; qkv projection itself runs in f32r from f32 x and weights.
"""

import sys

sys.path.insert(0, "/opt/trn_rl_repo")

from contextlib import ExitStack

import numpy as np

import concourse.bacc as bacc
import concourse.bass as bass
import concourse.mybir as mybir
import concourse.tile as tile

B, N, C, H, D = 2, 4096, 768, 12, 64
SCALE = D ** -0.5
F32 = mybir.dt.float32
BF16 = mybir.dt.bfloat16
I16 = mybir.dt.int16
MM_DT = mybir.dt.float32r

# DVE Schraudolph exp2 trick constants (bf16 bit layout: exponent at bit 7)
LOG2E = 1.4426950408889634
DVE_A = 128.0 * LOG2E
DVE_B = 127.0 * 128.0 - 7.2 + 0.5  # -c to zero mean rel err, +0.5 for floor

# key blocks whose exp runs on DVE instead of ACT: placed where the PE
# stream has extra inserted work (tail at kb6-7, qproj at kb30-31) so the
# slower DVE exp hides behind it
def _dve_kbs(NB):
    return frozenset(kb for kb in range(NB) if kb % 8 == 5)

# qkv weight column layout: m0 q01 | m1 k01 | m2 q2 | m3 k2 | m4 v01 | m5 v2
MOFF = [0, 128, 256, 320, 384, 512]
MW = [128, 128, 64, 64, 128, 64]
WCOLS = 576

Exp = mybir.ActivationFunctionType.Exp
Copy = mybir.ActivationFunctionType.Copy


def build_nc(seq=N):
    NS = seq // 512  # 512-wide query chunks
    NB = seq // 128  # 128-wide key blocks
    dve_kbs = _dve_kbs(NB)

    nc = bacc.Bacc("TRN2", target_bir_lowering=False, debug=False, num_devices=8)
    xt = nc.dram_tensor("xt", [768, seq], MM_DT, kind="ExternalInput").ap()
    wqkv = nc.dram_tensor("wqkv", [768, WCOLS], MM_DT, kind="ExternalInput").ap()
    wb = nc.dram_tensor("wb", [128, 6], F32, kind="ExternalInput").ap()
    pwt = nc.dram_tensor("pwt", [256, 768], F32, kind="ExternalInput").ap()
    ident = nc.dram_tensor("ident", [128, 192], F32, kind="ExternalInput").ap()
    y = nc.dram_tensor("y", [seq, 768], F32, kind="ExternalOutput").ap()

    with tile.TileContext(nc) as tc, ExitStack() as ctx:
        const = ctx.enter_context(tc.tile_pool(name="const", bufs=1))
        big = ctx.enter_context(tc.tile_pool(name="big", bufs=1))
        stg = ctx.enter_context(tc.tile_pool(name="stg", bufs=2))
        xs_pool = ctx.enter_context(tc.tile_pool(name="xs", bufs=18))
        pt_pool = ctx.enter_context(tc.tile_pool(name="pt", bufs=4))
        vst_pool = ctx.enter_context(tc.tile_pool(name="vst", bufs=4))
        onab_pool = ctx.enter_context(tc.tile_pool(name="onab", bufs=2))
        onc_pool = ctx.enter_context(tc.tile_pool(name="onc", bufs=2))
        otab_pool = ctx.enter_context(tc.tile_pool(name="otab", bufs=2))
        otc_pool = ctx.enter_context(tc.tile_pool(name="otc", bufs=2))
        dnm_pool = ctx.enter_context(tc.tile_pool(name="dnm", bufs=2))
        ysb_pool = ctx.enter_context(tc.tile_pool(name="ysb", bufs=4))
        sp = ctx.enter_context(tc.tile_pool(name="sp", bufs=2, space="PSUM"))
        oa = ctx.enter_context(tc.tile_pool(name="oa", bufs=2, space="PSUM"))

        # ---- constants ----
        w_sb = []
        for cch in range(6):
            t = const.tile([128, WCOLS], MM_DT, tag=f"w{cch}", name=f"w{cch}")
            nc.sync.dma_start(t[:], wqkv[cch * 128:(cch + 1) * 128, :])
            w_sb.append(t)
        wb_sb = const.tile([128, 6], F32, tag="wb")
        nc.sync.dma_start(wb_sb[:], wb[:])

        # proj weights + identity: DMA f32 staging, convert to bf16 once
        pws = stg.tile([128, 768], F32, tag="stg", name="pws_ab")
        nc.sync.dma_start(pws[:], pwt[0:128, :])
        pwab = const.tile([128, 768], BF16, tag="pwab")
        nc.vector.tensor_copy(pwab[:], pws[:])
        pwsc = stg.tile([128, 768], F32, tag="stg", name="pws_c")
        nc.sync.dma_start(pwsc[:], pwt[128:256, :])
        pwc = const.tile([64, 768], BF16, tag="pwc")
        nc.vector.tensor_copy(pwc[:], pwsc[0:64, :])
        ids = stg.tile([128, 192], F32, tag="ids", name="ids")
        nc.sync.dma_start(ids[:], ident[:])
        idb = const.tile([128, 192], BF16, tag="idb")
        nc.vector.tensor_copy(idb[:], ids[:])

        # ---- persistent qkv^T tensors (bf16) ----
        ka = big.tile([128, seq], BF16, tag="ka")
        kb_ = big.tile([128, seq], BF16, tag="kb")
        kc = big.tile([128, seq], BF16, tag="kc")
        q01 = big.tile([128, seq], BF16, tag="q01")
        q2 = big.tile([128, seq], BF16, tag="q2")
        nc.gpsimd.memset(ka[64:128, :], 0.0)
        nc.gpsimd.memset(kb_[0:64, :], 0.0)
        nc.gpsimd.memset(kc[64:128, :], 0.0)
        nc.gpsimd.memset(q2[64:128, :], 0.0)
        vaug = [
            big.tile([128, NB * 65], BF16, tag=f"va{h}", name=f"va{h}")
            for h in range(3)
        ]
        for h in range(3):
            nc.gpsimd.memset(vaug[h][:], 1.0)

        def qproj(Q):
            """Project q01/q2 for query chunk Q (12 matmuls + 2 copybacks)."""
            qs = slice(Q * 512, (Q + 1) * 512)
            xq = []
            for cch in range(6):
                t = xs_pool.tile([128, 512], MM_DT, tag="xs", name="xq")
                nc.sync.dma_start(t[:], xt[cch * 128:(cch + 1) * 128, qs])
                xq.append(t)
            spq = sp.tile([128, 1536], F32, tag="sp", name="spq")
            for m, co in ((0, 0), (2, 512)):
                w = MW[m]
                for cch in range(6):
                    nc.tensor.matmul(
                        spq[0:w, co:co + 512],
                        lhsT=w_sb[cch][:, MOFF[m]:MOFF[m] + w],
                        rhs=xq[cch][:],
                        start=(cch == 0),
                        stop=(cch == 5),
                    )
            nc.vector.tensor_scalar_add(q01[:, qs], spq[:, 0:512], wb_sb[:, 0:1])
            nc.vector.tensor_scalar_add(
                q2[0:64, qs], spq[0:64, 512:1024], wb_sb[0:64, 2:3]
            )

        def phase1(s):
            """k/v projection + v transpose for key chunk s (all via sp pool)."""
            ss = slice(s * 512, (s + 1) * 512)
            xs = []
            for cch in range(6):
                t = xs_pool.tile([128, 512], MM_DT, tag="xs", name="xs")
                nc.sync.dma_start(t[:], xt[cch * 128:(cch + 1) * 128, ss])
                xs.append(t)
            kps = sp.tile([128, 1536], F32, tag="sp", name="kps")
            for m, co in ((1, 0), (3, 512)):
                w = MW[m]
                for cch in range(6):
                    nc.tensor.matmul(
                        kps[0:w, co:co + 512],
                        lhsT=w_sb[cch][:, MOFF[m]:MOFF[m] + w],
                        rhs=xs[cch][:],
                        start=(cch == 0),
                        stop=(cch == 5),
                    )
            nc.vector.tensor_scalar_add(ka[0:64, ss], kps[0:64, 0:512],
                                        wb_sb[0:64, 1:2])
            nc.vector.tensor_scalar_add(kb_[64:128, ss], kps[64:128, 0:512],
                                        wb_sb[64:128, 1:2])
            nc.vector.tensor_scalar_add(kc[0:64, ss], kps[0:64, 512:1024],
                                        wb_sb[0:64, 3:4])
            vps = sp.tile([128, 1536], F32, tag="sp", name="vps")
            for m, co in ((4, 0), (5, 512)):
                w = MW[m]
                for cch in range(6):
                    nc.tensor.matmul(
                        vps[0:w, co:co + 512],
                        lhsT=w_sb[cch][:, MOFF[m]:MOFF[m] + w],
                        rhs=xs[cch][:],
                        start=(cch == 0),
                        stop=(cch == 5),
                    )
            vst01 = vst_pool.tile([128, 512], BF16, tag="vst01", name="vst01")
            nc.scalar.activation(vst01[:], vps[:, 0:512], Copy)
            vst2 = vst_pool.tile([64, 512], BF16, tag="vst2", name="vst2")
            nc.scalar.activation(vst2[:], vps[0:64, 512:1024], Copy)
            # transpose v into vaug [keys, d]: a|b via eye128 (cols split),
            # c via eye64.  One sp tile: a|b in cols 0:512, c in 1024:1280.
            tp = sp.tile([128, 1536], F32, tag="sp", name="tpv")
            for j in range(4):
                nc.tensor.matmul(
                    tp[:, j * 128:(j + 1) * 128],
                    lhsT=vst01[:, j * 128:(j + 1) * 128],
                    rhs=idb[:, 64:192],
                    start=True,
                    stop=True,
                )
                nc.tensor.matmul(
                    tp[:, 1024 + j * 64:1024 + (j + 1) * 64],
                    lhsT=vst2[:, j * 128:(j + 1) * 128],
                    rhs=idb[0:64, 0:64],
                    start=True,
                    stop=True,
                )
            for j in range(4):
                blk = 4 * s + j
                nc.vector.tensor_copy(
                    vaug[0][:, blk * 65:blk * 65 + 64],
                    tp[:, j * 128:j * 128 + 64],
                )
                nc.vector.tensor_copy(
                    vaug[1][:, blk * 65:blk * 65 + 64],
                    tp[:, j * 128 + 64:j * 128 + 128],
                )
                nc.scalar.activation(
                    vaug[2][:, blk * 65:blk * 65 + 64],
                    tp[:, 1024 + j * 64:1024 + j * 64 + 64], Copy,
                )

        # tail state carried across chunks for software pipelining
        tail = {}

        def emit_tail_pe(Q):
            """Transposes + proj + y DMA for chunk Q (emitted inside Q+1)."""
            onab, onc = tail.pop(Q)
            tpab = sp.tile([128, 1536], F32, tag="sp", name="tpab")
            for j in range(4):
                nc.tensor.matmul(
                    tpab[:, j * 128:(j + 1) * 128],
                    lhsT=onab[:, j * 128:(j + 1) * 128],
                    rhs=idb[:, 64:192],
                    start=True,
                    stop=True,
                )
            otab = otab_pool.tile([128, 512], BF16, tag="otab")
            nc.vector.tensor_copy(otab[:], tpab[:, 0:512])
            tpc = sp.tile([128, 1536], F32, tag="sp", name="tpc")
            for j in range(4):
                nc.tensor.matmul(
                    tpc[0:64, j * 128:(j + 1) * 128],
                    lhsT=onc[:, j * 64:(j + 1) * 64],
                    rhs=idb[:, 64:192],
                    start=True,
                    stop=True,
                )
            otc = otc_pool.tile([64, 512], BF16, tag="otc")
            nc.vector.tensor_copy(otc[:], tpc[0:64, 0:512])
            for j in range(4):
                psy = sp.tile([128, 1536], F32, tag="sp", name="psy")
                for co, cw in ((0, 512), (512, 256)):
                    nc.tensor.matmul(
                        psy[:, co:co + cw],
                        lhsT=otab[:, j * 128:(j + 1) * 128],
                        rhs=pwab[:, co:co + cw],
                        start=True,
                        stop=False,
                    )
                    nc.tensor.matmul(
                        psy[:, co:co + cw],
                        lhsT=otc[:, j * 128:(j + 1) * 128],
                        rhs=pwc[:, co:co + cw],
                        start=False,
                        stop=True,
                    )
                ysb = ysb_pool.tile([128, 768], F32, tag="ysb", name="ysb")
                nc.vector.tensor_copy(ysb[:], psy[:, 0:768])
                r0 = Q * 512 + j * 128
                nc.sync.dma_start(y[r0:r0 + 128, :], ysb[:])

        # ---- prologue: first key chunk + first q chunk ----
        phase1(0)
        qproj(0)

        # ---- main loop over query chunks ----
        for Q in range(NS):
            qs = slice(Q * 512, (Q + 1) * 512)
            oaA = oa.tile([128, 512], F32, tag="oa", name="oaA")
            oaB = oa.tile([128, 512], F32, tag="oa", name="oaB")

            def emit_av(kb, ptt, oaA=oaA, oaB=oaB):
                # One PSUM accumulation group per bank: start marks the whole
                # 2KB zero region; later windows' first writes overwrite from
                # pending-zero, so only window 0 starts and window 5 stops.
                for h in range(3):
                    for j in range(4):
                        idx = h * 4 + j
                        bank, pos = (oaA, idx) if idx < 6 else (oaB, idx - 6)
                        nc.tensor.matmul(
                            bank[:, pos * 65:pos * 65 + 65],
                            lhsT=ptt[:, h * 512 + j * 128:h * 512 + (j + 1) * 128],
                            rhs=vaug[h][:, kb * 65:(kb + 1) * 65],
                            start=(kb == 0 and pos == 0),
                            stop=(kb == NB - 1 and pos == 5),
                        )

            pend = []
            for kb in range(NB):
                spt = sp.tile([128, 1536], F32, tag="sp", name="spt")
                for h, (kt, qt) in enumerate(((ka, q01), (kb_, q01), (kc, q2))):
                    nc.tensor.matmul(
                        spt[:, h * 512:(h + 1) * 512],
                        lhsT=kt[:, kb * 128:(kb + 1) * 128],
                        rhs=qt[:, qs],
                        start=True,
                        stop=True,
                    )
                ptt = pt_pool.tile([128, 1536], BF16, tag="pt", name="pt")
                if kb in dve_kbs:
                    nc.vector.tensor_scalar(
                        ptt[:].bitcast(I16), spt[:], DVE_A, DVE_B,
                        mybir.AluOpType.mult, mybir.AluOpType.add,
                    )
                else:
                    nc.scalar.activation(ptt[:], spt[:], Exp)
                pend.append((kb, ptt))
                if len(pend) >= 3:
                    emit_av(*pend.pop(0))
                if Q == 0 and kb % 4 == 3 and kb < 4 * (NS - 1):
                    phase1(kb // 4 + 1)  # fuse remaining k/v chunks into Q0
                if Q > 0 and kb == 6:
                    emit_tail_pe(Q - 1)  # previous chunk's transposes + proj
            if Q + 1 < NS:
                qproj(Q + 1)
            for e in pend:
                emit_av(*e)

            # normalize at the chunk boundary: the next chunk's AV reuses
            # these oa banks (WAR), so split the 12 multiplies across DVE
            # (bank A) and ACT (bank B) to clear them within ~2 kbs.
            dnm = dnm_pool.tile([128, 16], F32, tag="dnm")
            nc.vector.reciprocal(dnm[:, 6:12], oaB[:, 64:64 + 6 * 65:65])
            nc.vector.reciprocal(dnm[:, 0:6], oaA[:, 64:64 + 6 * 65:65])
            onab = onab_pool.tile([128, 512], BF16, tag="onab")
            onc = onc_pool.tile([128, 256], BF16, tag="onc")
            for h in range(3):
                for j in range(4):
                    idx = h * 4 + j
                    bank, pos = (oaA, idx) if idx < 6 else (oaB, idx - 6)
                    if h < 2:
                        dst = onab[:, j * 128 + h * 64:j * 128 + (h + 1) * 64]
                    else:
                        dst = onc[:, j * 64:(j + 1) * 64]
                    if idx < 6:
                        nc.vector.tensor_scalar_mul(
                            dst, bank[:, pos * 65:pos * 65 + 64],
                            dnm[:, idx:idx + 1]
                        )
                    else:
                        nc.scalar.activation(
                            dst, bank[:, pos * 65:pos * 65 + 64], Copy,
                            scale=dnm[:, idx:idx + 1],
                        )
            tail[Q] = (onab, onc)
        emit_tail_pe(NS - 1)

    nc.compile()
    return nc


def host_prep(x, qkv_w, qkv_b, proj_w, seq=N):
    """Build the 8 per-core input maps."""
    f = np.float32
    x = np.asarray(x, f)
    qkv_w = np.asarray(qkv_w, f)
    qkv_b = np.asarray(qkv_b, f)
    proj_w = np.asarray(proj_w, f)

    xts = [np.ascontiguousarray(x[b].T) for b in range(B)]
    ident = np.zeros((128, 192), f)
    ident[0:64, 0:64] = np.eye(64, dtype=f)
    ident[64:128, 0:64] = np.eye(64, dtype=f)
    ident[:, 64:192] = np.eye(128, dtype=f)

    in_maps = []
    for core in range(8):
        b, g = core // 4, core % 4
        ha, hb_, hc = 3 * g, 3 * g + 1, 3 * g + 2

        def Wrow(base, h):
            return qkv_w[base + h * 64:base + (h + 1) * 64, :]  # [64, 768]

        def brow(base, h):
            return qkv_b[base + h * 64:base + (h + 1) * 64]

        cols = np.concatenate(
            [
                Wrow(0, ha).T * SCALE, Wrow(0, hb_).T * SCALE,  # m0 q01
                Wrow(C, ha).T, Wrow(C, hb_).T,                  # m1 k01
                Wrow(0, hc).T * SCALE,                          # m2 q2
                Wrow(C, hc).T,                                  # m3 k2
                Wrow(2 * C, ha).T, Wrow(2 * C, hb_).T,          # m4 v01
                Wrow(2 * C, hc).T,                              # m5 v2
            ],
            axis=1,
        )  # [768, 576]
        bias = np.concatenate(
            [
                brow(0, ha) * SCALE, brow(0, hb_) * SCALE,
                brow(C, ha), brow(C, hb_),
                brow(0, hc) * SCALE,
                brow(C, hc),
                brow(2 * C, ha), brow(2 * C, hb_), brow(2 * C, hc),
            ]
        )  # [576]
        wbm = np.zeros((128, 6), f)
        for m in range(6):
            wbm[0:MW[m], m] = bias[MOFF[m]:MOFF[m] + MW[m]]
        pwtm = np.zeros((256, 768), f)
        pwtm[0:64, :] = proj_w.T[ha * 64:(ha + 1) * 64, :]
        pwtm[64:128, :] = proj_w.T[hb_ * 64:(hb_ + 1) * 64, :]
        pwtm[128:192, :] = proj_w.T[hc * 64:(hc + 1) * 64, :]

        in_maps.append(
            {
                "xt": xts[b][:, :seq],
                "wqkv": np.ascontiguousarray(cols),
                "wb": wbm,
                "pwt": pwtm,
                "ident": ident,
            }
        )
    return in_maps


_nc_cache = {}


def _get_nc(seq=N):
    if seq not in _nc_cache:
        _nc_cache[seq] = build_nc(seq)
    return _nc_cache[seq]


def kernel(x, qkv_w, qkv_b, proj_w, proj_b, _trace=False):
    from concourse.bass_utils import run_bass_kernel_spmd

    nc = _get_nc()
    in_maps = host_prep(x, qkv_w, qkv_b, proj_w)
    res = run_bass_kernel_spmd(nc, in_maps, list(range(8)), trace=_trace)
    proj_b = np.asarray(proj_b, np.float32)
    out = np.zeros((B, N, C), np.float32)
    for b in range(B):
        acc = np.zeros((N, C), np.float32)
        for g in range(4):
            acc += res.results[b * 4 + g]["y"]
        out[b] = acc + proj_b[None, :]
    if _trace:
        return out, res
    return out
